# revision 3
# baseline (speedup 1.0000x reference)
"""Trainium2 Bass kernel for the RNN-T JointNetwork problem.

Computes log_softmax(tanh(cat(enc, pred)) @ W.T + b) over the vocab dim
for logits of shape [B=4, T=200, U=50, V=1024], fp32.

Data-parallel over the 800 flattened (b,t) rows, 100 per core, split in
two shards of 50 t's so ONE combined one-hot stationary operand performs
both the u-broadcast and the t-broadcast in a single matmul pass
(K = 50 pred rows + 50 enc rows = 100 <= 128).

Per core:
  fat warmup matmuls (full 128x128 array) while inputs load -- the PE
    p-state ramps on array utilization and then survives multi-us gaps
  enc_p  = tanh(e) @ We.T   [100, 1024] -> f16 -> comb rows 50:99
  pred_b = tanh(p) @ Wp.T + b  [50, 1024] -> f16 -> comb rows 0:49
  log-sum-exp via a tiny GEMM instead of per-tile exp reductions:
    EeT = exp(enc_p.T) (8 PE transposes), EpT = exp(Wp.T tanh(p) + b)
    (direct GEMM-T, bias folded into the Exp's per-partition bias),
    s[t, u] = sum_v EeT EpT (8 matmuls), nlse = -ln(s)   [100, 50]
  nlse -> DRAM -> reload as per-row-tile columns [128, 40]
  main loop (40 row-tiles of 128 rows, r = t*50 + u within shard):
    x    = ind[k].T @ comb[shard]    (one matmul per 512-wide v chunk)
    o    = x + nlse[:, k]            (f32 PSUM -> f16 SBUF, whole tile on
                                      ONE engine, DVE/ACT alternating --
                                      two writers on one tile serialize)
    DMA out tile -> DRAM f16 (host casts back to f32)

Scheduling notes (measured on HW): every dma_start costs ~0.7us of its
queue engine and queues want >=2KB contiguous descriptor runs, so all
host inputs are pre-rearranged to contiguous per-partition lines and W
is split across three queues; the activation tables are patched so the
one table switch (tanh set -> exp/ln set) lands in an ACT idle window.
"""

import numpy as np

import concourse.bass as bass
import concourse.bacc as bacc
import concourse.tile as tile
from concourse import mybir
from concourse.bass_utils import run_bass_kernel_spmd

B, T, U, D, V = 4, 200, 50, 512, 1024
N_CORES = 8
BT = B * T
TPC = BT // N_CORES            # 100 (b,t) rows per core
SH = 2
TPS = TPC // SH                # 50 t's per shard
RPS = TPS * U                  # 2500 rows per shard
ROWS = TPC * U                 # 5000 rows per core
P = 128
NTS = (RPS + P - 1) // P       # 20 tiles per shard
NT = SH * NTS                  # 40 tiles per core
NV = V // 512
DC = D // P                    # 4
VC = V // P                    # 8

f32 = mybir.dt.float32
f16 = mybir.dt.float16

TRACE = False
LAST_RESULT = None

_CACHE = {}

# postprocess engine per tile (mod 5): GPSIMD cannot read PSUM, so split
# DVE/ACT by measured per-op cost (DVE ~1535ns, ACT ~1030ns) -> 2:3
_PP = (["dve", "act"] * 9 + ["act", "act"])


def _patch_act_tables():
    """Two table loads total: (tanh, exp) in exp_and_others, then
    (ln, identity, copy) in natural_log_exp_and_others."""
    if getattr(bacc, "_joint_act_patch8", False):
        return
    orig = bacc.get_activation_tables

    def patched(arch):
        t = dict(orig(arch))
        A = mybir.ActivationFunctionType
        for name, fns in t.items():
            fns = set(fns)
            if name != "exp_and_others":
                fns -= {A.Tanh}
            if name != "natural_log_exp_and_others":
                fns -= {A.Exp, A.Ln}
            t[name] = fns
        return t

    bacc.get_activation_tables = patched
    bacc._joint_act_patch8 = True


def _build_indicators():
    ind = np.zeros((TPC, NT, P), dtype=np.float16)
    for k in range(NT):
        r = 128 * (k % NTS) + np.arange(P)
        valid = r < RPS
        rv = r[valid]
        cv = np.arange(P)[valid]
        ind[rv % U, k, cv] = 1.0
        ind[U + rv // U, k, cv] = 1.0
    return np.ascontiguousarray(ind.reshape(TPC, NT * P))


def _build_program():
    _patch_act_tables()
    nc = bacc.Bacc("TRN2", target_bir_lowering=False, debug=False,
                   num_devices=N_CORES)

    encT = nc.dram_tensor("enc_r", [P, DC * TPC], f32, kind="ExternalInput")
    predT = nc.dram_tensor("pred_r", [P, DC * U], f32, kind="ExternalInput")
    wT = nc.dram_tensor("w_r", [P, 2 * DC * V], f16, kind="ExternalInput")
    bias_row = nc.dram_tensor("bias_row", [1, V], f16, kind="ExternalInput")
    bias_col = nc.dram_tensor("bias_col", [P, VC], f32, kind="ExternalInput")
    ident_in = nc.dram_tensor("ident", [P, P], f16, kind="ExternalInput")
    ind = nc.dram_tensor("ind", [TPC, NT * P], f16, kind="ExternalInput")
    out = nc.dram_tensor("out", [ROWS, V], f16, kind="ExternalOutput")
    lse_dram = nc.dram_tensor("lse_scratch", [NT * P], f32, kind="Internal")

    with tile.TileContext(nc) as tc:
        with (
            tc.tile_pool(name="consts", bufs=1) as consts,
            tc.tile_pool(name="pbig", bufs=3, space=bass.MemorySpace.PSUM) as pbig,
            tc.tile_pool(name="psm", bufs=2, space=bass.MemorySpace.PSUM) as psm,
            tc.tile_pool(name="outs", bufs=8) as outs,
            tc.tile_pool(name="small", bufs=4) as small,
        ):
            # ---- input DMAs (host-rearranged: contiguous 2KB+ lines) ----
            encT_sb = consts.tile([P, DC, TPC], f32)
            nc.sync.dma_start(out=encT_sb[:], in_=encT.ap().rearrange(
                "p (c t) -> p c t", c=DC))
            predT_sb = consts.tile([P, DC, U], f32)
            nc.scalar.dma_start(out=predT_sb[:], in_=predT.ap().rearrange(
                "p (c u) -> p c u", c=DC))

            wt_sb = consts.tile([P, 2 * DC, V], f16)
            wT_r = wT.ap().rearrange("p (c v) -> p c v", c=2 * DC)
            nc.sync.dma_start(out=wt_sb[:, 0:2, :], in_=wT_r[:, 0:2, :])
            nc.scalar.dma_start(out=wt_sb[:, 2:4, :], in_=wT_r[:, 2:4, :])
            nc.gpsimd.dma_start(out=wt_sb[:, 4:6, :], in_=wT_r[:, 4:6, :])
            nc.gpsimd.dma_start(out=wt_sb[:, 6:8, :], in_=wT_r[:, 6:8, :])

            brow_sb = consts.tile([1, V], f16)
            nc.gpsimd.dma_start(out=brow_sb[:], in_=bias_row.ap())
            bcol_sb = consts.tile([P, VC], f32)
            nc.gpsimd.dma_start(out=bcol_sb[:], in_=bias_col.ap())
            ident = consts.tile([P, P], f16)
            nc.gpsimd.dma_start(out=ident[:], in_=ident_in.ap())

            ind_sb = consts.tile([TPC, NT, P], f16)
            nc.scalar.dma_start(
                out=ind_sb[:],
                in_=ind.ap().rearrange("q (k p) -> q k p", k=NT))

            ones_u = consts.tile([1, U], f16)
            nc.vector.memset(ones_u[:], 1.0)
            # fat warmups: the PE p-state ramps on array utilization, so
            # K=1 matmuls never trigger it -- use full [128,128]x[128,512]
            warm_a = consts.tile([P, P], f16)
            nc.vector.memset(warm_a[:], 0.25)
            warm_b = consts.tile([P, 512], f16)
            nc.vector.memset(warm_b[:], 0.25)
            for w in range(17):
                warm_ps = psm.tile([P, 512], f32, tag="s")
                nc.tensor.matmul(warm_ps[:], warm_a[:], warm_b[:],
                                 start=True, stop=True)

            # ---- tanh ----
            teTp = consts.tile([P, DC, TPC + U], f16)
            nc.scalar.activation(teTp[:, :, 0:TPC], encT_sb[:],
                                 mybir.ActivationFunctionType.Tanh)
            nc.scalar.activation(teTp[:, :, TPC:TPC + U], predT_sb[:],
                                 mybir.ActivationFunctionType.Tanh)

            # ---- enc_p / pred_b GEMMs ----
            enc_ps = pbig.tile([TPC, V], f32, tag="x")
            for sl_i in range(NV):
                sl = slice(sl_i * 512, (sl_i + 1) * 512)
                for c in range(DC):
                    nc.tensor.matmul(enc_ps[:, sl], teTp[:, c, 0:TPC],
                                     wt_sb[:, c, sl],
                                     start=(c == 0), stop=(c == DC - 1))
            pred_ps = pbig.tile([U, V], f32, tag="x")
            for sl_i in range(NV):
                sl = slice(sl_i * 512, (sl_i + 1) * 512)
                for c in range(DC):
                    nc.tensor.matmul(pred_ps[:, sl], teTp[:, c, TPC:TPC + U],
                                     wt_sb[:, DC + c, sl],
                                     start=(c == 0), stop=False)
                nc.tensor.matmul(pred_ps[:, sl], ones_u[:], brow_sb[:, sl],
                                 start=False, stop=True)

            # ---- casts to f16: comb rows + enc_f16 ----
            comb_a = consts.tile([TPC, V], f16)
            comb_b = consts.tile([TPC, V], f16)
            comb = [comb_a, comb_b]
            enc_f16 = consts.tile([TPC, V], f16)
            nc.vector.tensor_copy(enc_f16[:], enc_ps[:])
            nc.scalar.activation(comb_a[0:U, :], pred_ps[:],
                                 mybir.ActivationFunctionType.Copy)
            nc.vector.tensor_copy(comb_b[0:U, :], comb_a[0:U, :])
            # partition-shifting copies must go through DMA
            nc.gpsimd.dma_start(out=comb_a[U:TPC, :], in_=enc_f16[0:TPS, :])
            nc.gpsimd.dma_start(out=comb_b[U:TPC, :],
                                in_=enc_f16[TPS:TPC, :])

            # ---- EeT via transposes; EpT via GEMM-T with Exp bias ----
            EeT = consts.tile([P, VC, TPC], f16)
            EpT = consts.tile([P, VC, U], f16)
            for vc in range(VC):
                vs = slice(vc * P, (vc + 1) * P)
                pool = psm if vc % 2 == 0 else pbig
                tag = "s" if vc % 2 == 0 else "x"
                tp_ps = pool.tile([P, U], f32, tag=tag)
                for c in range(DC):
                    nc.tensor.matmul(tp_ps[:], wt_sb[:, DC + c, vs],
                                     teTp[:, c, TPC:TPC + U],
                                     start=(c == 0), stop=(c == DC - 1))
                nc.scalar.activation(EpT[:, vc, :], tp_ps[:],
                                     mybir.ActivationFunctionType.Exp,
                                     bias=bcol_sb[:, vc:vc + 1])
            for vc in range(VC):
                vs = slice(vc * P, (vc + 1) * P)
                pool = psm if vc % 2 == 0 else pbig
                tag = "s" if vc % 2 == 0 else "x"
                te_ps = pool.tile([P, TPC], f16, tag=tag)
                nc.tensor.transpose(te_ps[:], enc_f16[:, vs],
                                    ident[0:TPC, 0:TPC])
                nc.scalar.activation(EeT[:, vc, :], te_ps[:],
                                     mybir.ActivationFunctionType.Exp)

            # ---- s = EeT.T @ EpT ; nlse = -ln(s) ; DRAM roundtrip ----
            s_ps = psm.tile([TPC, U], f32, tag="s")
            for vc in range(VC):
                nc.tensor.matmul(s_ps[:], EeT[:, vc, :], EpT[:, vc, :],
                                 start=(vc == 0), stop=(vc == VC - 1))
            lse_sb = small.tile([TPC, U], f32)
            nc.scalar.activation(lse_sb[:], s_ps[:],
                                 mybir.ActivationFunctionType.Ln)
            nlse_sb = small.tile([TPC, U], f32)
            nc.vector.tensor_scalar_mul(nlse_sb[:], lse_sb[:], -1.0)

            for sh in range(SH):
                dst = lse_dram.ap()[sh * NTS * P: sh * NTS * P + RPS]
                nc.sync.dma_start(
                    out=dst.rearrange("(t u) -> t u", t=TPS),
                    in_=nlse_sb[sh * TPS:(sh + 1) * TPS, :])
            nlse_rows = consts.tile([P, NT], f32)
            lse_r = lse_dram.ap().rearrange("(k p) -> p k", p=P)
            KQ = NT // 8
            engs = [nc.sync, nc.gpsimd, nc.scalar]
            for q in range(8):
                ks = slice(q * KQ, (q + 1) * KQ)
                engs[q % 3].dma_start(out=nlse_rows[:, ks], in_=lse_r[:, ks])

            # ---- main loop ----
            for k in range(NT):
                sh = k // NTS
                r_loc = P * (k % NTS)
                rows = min(P, RPS - r_loc)
                r0 = sh * RPS + r_loc
                x_ps = pbig.tile([P, V], f32, tag="x")
                for sl_i in range(NV):
                    sl = slice(sl_i * 512, (sl_i + 1) * 512)
                    nc.tensor.matmul(x_ps[:, sl], ind_sb[:, k, :],
                                     comb[sh][:, sl],
                                     start=True, stop=True)
                if k == 2:
                    # bridge the PE stall until the first postprocess frees
                    # a PSUM slot; fat warmups keep the p-state ramp alive
                    for w in range(10):
                        warm_ps = psm.tile([P, 512], f32, tag="s")
                        nc.tensor.matmul(warm_ps[:], warm_a[:], warm_b[:],
                                         start=True, stop=True)
                o = outs.tile([P, V], f16)
                pp = _PP[k % len(_PP)]
                col = nlse_rows[:rows, k:k + 1]
                if pp == "dve":
                    nc.vector.tensor_scalar_add(o[:rows], x_ps[:rows], col)
                else:
                    nc.scalar.activation(
                        o[:rows], x_ps[:rows],
                        mybir.ActivationFunctionType.Identity, bias=col)
                nc.gpsimd.dma_start(out=out.ap()[r0:r0 + rows, :],
                                    in_=o[:rows])

    nc.compile()
    return nc


def kernel(enc_out, pred_out, W, b):
    global LAST_RESULT
    enc_out = np.asarray(enc_out, dtype=np.float32)
    pred_out = np.asarray(pred_out, dtype=np.float32)
    W = np.asarray(W, dtype=np.float32)
    b = np.asarray(b, dtype=np.float32)

    if "nc" not in _CACHE:
        _CACHE["nc"] = _build_program()
        _CACHE["ind"] = _build_indicators()
    nc = _CACHE["nc"]
    ind = _CACHE["ind"]

    wTh = W.T.astype(np.float16)
    w_r = np.ascontiguousarray(
        wTh.reshape(2 * DC, P, V).transpose(1, 0, 2).reshape(P, 2 * DC * V))
    bias_row = b.reshape(1, V).astype(np.float16)
    bias_col = np.ascontiguousarray(b.reshape(VC, P).T).astype(np.float32)
    ident = np.eye(P, dtype=np.float16)
    enc_flat = enc_out.reshape(BT, D)

    in_maps = []
    for c in range(N_CORES):
        bt0 = c * TPC
        b_idx = bt0 // T
        encT_l = enc_flat[bt0:bt0 + TPC].T
        enc_rr = np.ascontiguousarray(
            encT_l.reshape(DC, P, TPC).transpose(1, 0, 2).reshape(
                P, DC * TPC))
        predT_l = pred_out[b_idx].T
        pred_rr = np.ascontiguousarray(
            predT_l.reshape(DC, P, U).transpose(1, 0, 2).reshape(P, DC * U))
        in_maps.append({
            "enc_r": enc_rr,
            "pred_r": pred_rr,
            "w_r": w_r,
            "bias_row": bias_row,
            "bias_col": bias_col,
            "ident": ident,
            "ind": ind,
        })

    res = run_bass_kernel_spmd(nc, in_maps, core_ids=list(range(N_CORES)),
                               trace=TRACE)
    LAST_RESULT = res
    full = np.concatenate([r["out"] for r in res.results], axis=0)
    return full.reshape(B, T, U, V).astype(np.float32)
